# revision 2
# baseline (speedup 1.0000x reference)
"""Trainium2 Bass kernel for nn_AttentionLayer (additive/Bahdanau-style attention).

Reference computation:
  qp = query @ W1[:H] ; kp = key @ W1[H:]          # [B,S1,HM], [B,S2,HM]
  hid = relu(qp[:,:,None,:] + kp[:,None,:,:] + b1)  # [B,S1,S2,HM]
  scores = hid @ W2 + b2                            # [B,S1,S2]
  A = where(qmask*kmask==0, 0, exp(scores))
  out = (A / clip(A.sum(-1), 2e-15)) @ value        # [B,S1,H]

Sharding: data-parallel over batch, 2 batches per core on 8 cores.

Sparsity: masked q rows produce all-zero output rows and masked k columns
contribute exactly zero to every sum, so the host compacts both dimensions
per batch (keeping only mask==1 rows, zero-padded to per-SLOT capacities)
and scatters the result back. Batches are sorted by kept-q count and paired
big-with-small across the two per-core slots, so slot capacities hug the
actual counts. The device still applies the k-mask (padding lanes carry
mask 0), so results are exact up to bf16 rounding of W1/query/key/hid.

Device mapping (per core, 2 batch slots):
  stage0 (PE):   qpT[h,q], kpT[h,k] projections (+b1 via ACT Identity evict)
  pair  (DVE/ACT): hidT[h,k] = relu(kpT_hb + qpT_hb[:,q]) per (slot,q,hb) as
                 one fused per-partition-scalar op; both slots write one wide
                 [128, K0+K1] tile consumed by a single PE matmul
  scores (PE):   matmul with sliding-window weights Z(hb,j) = zmr[:,31-j:63-j]
                 (W2 h-block parked at column 31) -> scores row lands on PSUM
                 partition q=32*cg+j via tile_position col groups, fp32 accum
  post (per slot): Exp(+b2); PE transpose -> A_T[k,q]; k_mask on PSUM evict;
                 out_un = A_T.T @ [value | ones] (ones column = row-sum);
                 out = out_un * 1/clip(rowsum, 2e-15) per-partition.
"""

import os
import sys

import numpy as np

for _p in ("/opt/trn_rl_repo",):
    if os.path.isdir(_p) and _p not in sys.path:
        sys.path.insert(0, _p)

import ml_dtypes  # noqa: E402
import concourse.bass as bass  # noqa: E402
import concourse.mybir as mybir  # noqa: E402
import concourse.tile as tile  # noqa: E402
from concourse import bacc  # noqa: E402
from concourse.bass_utils import run_bass_kernel_spmd  # noqa: E402
from concourse.masks import make_identity  # noqa: E402

B, S1, S2, H, HM = 16, 128, 256, 256, 512
N_CORES = 8
BPC = B // N_CORES  # batch slots per core
NHB = HM // 128  # h blocks
NDC = H // 128  # d chunks (projection contraction)
VA = H + 2  # value dims + ones column + pad
ACT_NUM, ACT_DEN = 43, 128  # pair-op share on the scalar engine

FP32 = mybir.dt.float32
F32R = mybir.dt.float32r
BF16 = mybir.dt.bfloat16
ADD = mybir.AluOpType.add
MAX = mybir.AluOpType.max
RELU = mybir.ActivationFunctionType.Relu
EXP = mybir.ActivationFunctionType.Exp
IDENT_F = mybir.ActivationFunctionType.Identity

_cache: dict = {}


def _build(QN, KK):
    """Build + compile the per-core program.

    QN/KK: per-slot q and k capacities, e.g. QN=(80, 64), KK=(144, 136).
    """
    ck = (QN, KK)
    if ck in _cache:
        return _cache[ck]

    CGC = [(q + 31) // 32 for q in QN]  # col groups per slot
    QP32 = [c * 32 for c in CGC]
    KC = [(k + 127) // 128 for k in KK]  # k chunks per slot
    KOFF = [0, KK[0]]  # slot column offsets in wide tiles
    KW = KK[0] + KK[1]  # wide k extent
    KCT = sum(KC)
    QTW = NDC * (QN[0] + QN[1])  # packed qT width
    KTW = NDC * (KK[0] + KK[1])  # packed kT width

    nc = bacc.Bacc("TRN2", target_bir_lowering=False, debug=False, num_devices=1)

    qT = nc.dram_tensor("qT", [128, QTW], BF16, kind="ExternalInput")
    kT = nc.dram_tensor("kT", [128, KTW], BF16, kind="ExternalInput")
    vaug = nc.dram_tensor("vaug", [128, KCT * VA], FP32, kind="ExternalInput")
    w1 = nc.dram_tensor("w1", [2 * H, HM], BF16, kind="ExternalInput")
    misc = nc.dram_tensor("misc", [128, NHB + KCT + 1], FP32, kind="ExternalInput")
    zmr = nc.dram_tensor("zmr", [128, NHB * 64], BF16, kind="ExternalInput")
    y = nc.dram_tensor("y", [BPC, QP32[0], H], FP32, kind="ExternalOutput")

    with tile.TileContext(nc) as tc:
        with (
            tc.tile_pool(name="const", bufs=1) as cp,
            tc.tile_pool(name="acts", bufs=1) as ap,
            tc.tile_pool(name="hid", bufs=160) as hp,
            tc.tile_pool(name="small", bufs=2) as sp,
            tc.tile_pool(name="psA", bufs=2, space=bass.MemorySpace.PSUM) as psA,
            tc.tile_pool(name="psS", bufs=1, space=bass.MemorySpace.PSUM) as psS,
            tc.tile_pool(name="psB", bufs=2, space=bass.MemorySpace.PSUM) as psB,
        ):
            # ---------------- inputs to SBUF ----------------
            # Warmup: trigger the ACT table load immediately (no DMA deps) so
            # the ~1.3us PSEUDO_LOAD_ACT_FUNC_SET overlaps the input DMAs.
            warm = cp.tile([1, 2], FP32, name="warm", tag="warm")
            nc.vector.memset(warm[:], 0.0)
            nc.scalar.activation(warm[:], warm[:], RELU)
            # Few wide DMAs (HWDGE issue is ~0.6us each on the sync
            # sequencer); critical path (w1k/kT -> first projections) first,
            # big loads split in halves so two queues stream in parallel.
            w1t = {}
            misc_all = cp.tile([128, NHB + KCT + 1], FP32, name="misc_all", tag="misc_all")
            nc.sync.dma_start(misc_all[:], misc.ap())
            qT_all = cp.tile([128, QTW], BF16, name="qT_all", tag="qT_all")
            nc.sync.dma_start(qT_all[:], qT.ap())
            tk0 = cp.tile([128, HM], BF16, name="w1ks0", tag="w1ks0")
            nc.gpsimd.dma_start(tk0[:, : HM // 2], w1.ap()[H : H + 128, : HM // 2])
            nc.gpsimd.dma_start(tk0[:, HM // 2 :], w1.ap()[H : H + 128, HM // 2 :])
            w1t["k", 0] = tk0
            kT_all = cp.tile([128, KTW], BF16, name="kT_all", tag="kT_all")
            hw = KTW // 2
            nc.gpsimd.dma_start(kT_all[:, :hw], kT.ap()[:, :hw])
            nc.gpsimd.dma_start(kT_all[:, hw:], kT.ap()[:, hw:])
            tk1 = cp.tile([128, HM], BF16, name="w1ks1", tag="w1ks1")
            nc.gpsimd.dma_start(tk1[:, : HM // 2], w1.ap()[H + 128 : H + 256, : HM // 2])
            nc.gpsimd.dma_start(tk1[:, HM // 2 :], w1.ap()[H + 128 : H + 256, HM // 2 :])
            w1t["k", 1] = tk1
            for dc in range(NDC):
                tq = cp.tile([128, HM], BF16, name=f"w1qs{dc}", tag=f"w1qs{dc}")
                nc.sync.dma_start(tq[:, : HM // 2], w1.ap()[dc * 128 : (dc + 1) * 128, : HM // 2])
                nc.sync.dma_start(tq[:, HM // 2 :], w1.ap()[dc * 128 : (dc + 1) * 128, HM // 2 :])
                w1t["q", dc] = tq
            zm = cp.tile([128, NHB * 64], BF16, name="zm", tag="zm")
            nc.sync.dma_start(zm[:], zmr.ap())
            va_all = cp.tile([128, KCT * VA], F32R, name="va_all", tag="va_all")
            vw = (KCT * VA) // 2
            nc.gpsimd.dma_start(va_all[:, :vw], vaug.ap()[:, :vw])
            nc.gpsimd.dma_start(va_all[:, vw:], vaug.ap()[:, vw:])
            ident = cp.tile([128, 128], FP32, name="ident", tag="ident")
            make_identity(nc, ident[:])

            w1q, w1k, qT_sb, kT_sb, va_sb, km_sb, b1_sb = {}, {}, {}, {}, {}, {}, {}
            qoff = koff = 0
            for b in range(BPC):
                for dc in range(NDC):
                    kT_sb[b, dc] = kT_all[:, koff : koff + KK[b]]
                    qT_sb[b, dc] = qT_all[:, qoff : qoff + QN[b]]
                    qoff += QN[b]
                    koff += KK[b]
            coff = 0
            for b in range(BPC):
                for kc in range(KC[b]):
                    va_sb[b, kc] = va_all[:, coff * VA : (coff + 1) * VA]
                    km_sb[b, kc] = misc_all[:, NHB + coff : NHB + coff + 1]
                    coff += 1
            for hb in range(NHB):
                b1_sb[hb] = misc_all[:, hb : hb + 1]
            b2_sb = misc_all[:, NHB + KCT : NHB + KCT + 1]
            for dc in range(NDC):
                for hb in range(NHB):
                    w1q[dc, hb] = w1t["q", dc][:, hb * 128 : (hb + 1) * 128]
                    w1k[dc, hb] = w1t["k", dc][:, hb * 128 : (hb + 1) * 128]

            # ---------------- stage 0: projections ----------------
            qpT, kpB = {}, {}
            for hb in range(NHB):
                for b in range(BPC):
                    ps = psA.tile([128, max(KK)], FP32, name="proj", tag="proj")
                    for dc in range(NDC):
                        nc.tensor.matmul(
                            ps[:, : QN[b]],
                            w1q[dc, hb],
                            qT_sb[b, dc],
                            start=(dc == 0),
                            stop=(dc == NDC - 1),
                        )
                    t = ap.tile([128, QN[b]], FP32, name=f"qpT{b}{hb}", tag=f"qpT{b}{hb}")
                    nc.scalar.activation(t[:], ps[:, : QN[b]], IDENT_F, bias=b1_sb[hb])
                    qpT[b, hb] = t
                    ps2 = psA.tile([128, max(KK)], FP32, name="proj", tag="proj")
                    for dc in range(NDC):
                        nc.tensor.matmul(
                            ps2[:, : KK[b]],
                            w1k[dc, hb],
                            kT_sb[b, dc],
                            start=(dc == 0),
                            stop=(dc == NDC - 1),
                        )
                    t2 = ap.tile([128, KK[b]], BF16, name=f"kpB{b}{hb}", tag=f"kpB{b}{hb}")
                    nc.scalar.activation(t2[:], ps2[:, : KK[b]], IDENT_F, bias=b1_sb[hb])
                    kpB[b, hb] = t2

            # ---------------- pair stage + score reduce ----------------
            scores = psS.tile([128, KW], FP32, name="scps", tag="scps")
            cnt = 0
            for hb in range(NHB):
                for j in range(32):
                    w_ap = zm[:, hb * 64 + (31 - j) : hb * 64 + (63 - j)]
                    for cg in range(CGC[0]):
                        q = cg * 32 + j
                        if q >= QN[0]:
                            continue
                        hid = hp.tile([128, KW], BF16, name="hid", tag="hid")
                        nw = KK[0]
                        for b in range(BPC):
                            if q >= QN[b]:
                                continue
                            nw = KOFF[b] + KK[b]
                            qcol = qpT[b, hb][:, q : q + 1]
                            dst = hid[:, KOFF[b] : KOFF[b] + KK[b]]
                            if (cnt * ACT_NUM) % ACT_DEN < ACT_NUM:
                                nc.scalar.activation(
                                    dst, kpB[b, hb][:], RELU, bias=qcol, scale=1.0
                                )
                            else:
                                nc.vector.tensor_scalar(
                                    dst, kpB[b, hb][:], qcol, 0.0, ADD, MAX
                                )
                            cnt += 1
                        jlast = min(31, QN[0] - 1 - cg * 32)
                        nc.tensor.matmul(
                            scores[cg * 32 : (cg + 1) * 32, :nw],
                            w_ap,
                            hid[:, :nw],
                            start=(hb == 0 and j == 0),
                            stop=(hb == NHB - 1 and j == jlast),
                            tile_position=(0, cg * 32),
                        )

            # ---------------- post: exp / transpose / mask / value ----------------
            # Phase-interleaved across the two slots so one slot's serial chain
            # overlaps the other's on different engines.
            A, AT, pso = {}, {}, {}
            for b in range(BPC):
                A[b] = ap.tile([128, KK[b]], FP32, name=f"Aexp{b}", tag=f"Aexp{b}")
                nc.scalar.activation(
                    A[b][: QP32[b], :],
                    scores[: QP32[b], KOFF[b] : KOFF[b] + KK[b]],
                    EXP,
                    bias=b2_sb[0 : QP32[b], :],
                    scale=1.0,
                )
            for b in range(BPC):
                for kc in range(KC[b]):
                    kw = min(128, KK[b] - kc * 128)
                    pst = psB.tile([128, QP32[b]], FP32, name="trps", tag="trps")
                    nc.tensor.transpose(
                        pst[:kw, :],
                        A[b][: QP32[b], kc * 128 : kc * 128 + kw],
                        ident[: QP32[b], : QP32[b]],
                    )
                    at = ap.tile([128, QP32[b]], F32R, name=f"AT{b}{kc}", tag=f"AT{b}{kc}")
                    nc.scalar.activation(
                        at[:kw, :], pst[:kw, :], IDENT_F, scale=km_sb[b, kc][0:kw, :]
                    )
                    AT[b, kc] = at
            for b in range(BPC):
                pso[b] = psB.tile([128, VA], FP32, name=f"oun{b}", tag=f"oun{b}", bufs=1)
                for kc in range(KC[b]):
                    kw = min(128, KK[b] - kc * 128)
                    nc.tensor.matmul(
                        pso[b][: QP32[b], :],
                        AT[b, kc][:kw, :],
                        va_sb[b, kc][0:kw, :],
                        start=(kc == 0),
                        stop=(kc == KC[b] - 1),
                    )
            for b in range(BPC):
                qp32 = QP32[b]
                asum = sp.tile([128, 1], FP32, name="asum", tag="asum")
                nc.vector.tensor_scalar_max(asum[:qp32, :], pso[b][:qp32, H : H + 1], 2e-15)
                rec = sp.tile([128, 1], FP32, name="rec", tag="rec")
                nc.vector.reciprocal(rec[:qp32, :], asum[:qp32, :])
                outt = ap.tile([128, H], FP32, name=f"out{b}", tag=f"out{b}")
                if b == 0:
                    nc.scalar.activation(
                        outt[:qp32, :], pso[b][:qp32, 0:H], IDENT_F, scale=rec[:qp32, 0:1]
                    )
                else:
                    nc.vector.tensor_scalar_mul(
                        outt[:qp32, :], pso[b][:qp32, 0:H], rec[:qp32, 0:1]
                    )
                if b == 0:
                    nc.sync.dma_start(y.ap()[b, 0:qp32, :], outt[:qp32, :])
                else:
                    nc.gpsimd.dma_start(y.ap()[b, 0:qp32, :], outt[:qp32, :])

    nc.compile()
    _cache[ck] = nc
    return nc


def _r(x, m):
    return ((max(int(x), 1) + m - 1) // m) * m


def _prep(query, key, value, q_mask, k_mask, W1, b1, W2, b2):
    query = np.asarray(query, np.float32)
    key = np.asarray(key, np.float32)
    value = np.asarray(value, np.float32)
    q_mask = np.asarray(q_mask, np.float32)
    k_mask = np.asarray(k_mask, np.float32)
    W1 = np.ascontiguousarray(np.asarray(W1, ml_dtypes.bfloat16))
    b1 = np.asarray(b1, np.float32)
    W2 = np.asarray(W2, np.float32)
    b2 = np.asarray(b2, np.float32)

    q_idx = [np.nonzero(q_mask[i] != 0)[0] for i in range(B)]
    k_idx = [np.nonzero(k_mask[i] != 0)[0] for i in range(B)]
    qn = np.array([len(ix) for ix in q_idx])

    # Slot assignment: big batches -> slot 0, small -> slot 1.  Try three
    # sort keys and keep the one minimizing padded pair-work.
    kn = np.array([len(ix) for ix in k_idx])

    def mk(order):
        sb = [list(order[:N_CORES]), list(order[N_CORES:])]
        q = tuple(_r(max(len(q_idx[i]) for i in sb[s]), 2) for s in range(BPC))
        k = tuple(_r(max(len(k_idx[i]) for i in sb[s]), 2) for s in range(BPC))
        return sb, q, k, (q[0] + q[1]) * (k[0] + k[1])

    cands = [mk(np.argsort(-key, kind="stable")) for key in (qn, kn, qn * 1000 + kn)]
    slot_batches, QN, KK, _ = min(cands, key=lambda t: t[3])
    KC = [(k + 127) // 128 for k in KK]
    KCT = sum(KC)

    zmr = np.zeros((128, NHB * 64), np.float32)
    for hb in range(NHB):
        zmr[:, hb * 64 + 31] = W2[hb * 128 : (hb + 1) * 128, 0]
    zmr = zmr.astype(ml_dtypes.bfloat16)

    assign = {}  # (core, slot) -> global batch idx
    in_maps = []
    QTW = NDC * (QN[0] + QN[1])
    KTW = NDC * (KK[0] + KK[1])
    for c in range(N_CORES):
        qTp = np.zeros((128, QTW), ml_dtypes.bfloat16)
        kTp = np.zeros((128, KTW), ml_dtypes.bfloat16)
        vap = np.zeros((128, KCT * VA), np.float32)
        miscp = np.zeros((128, NHB + KCT + 1), np.float32)
        miscp[:, :NHB] = b1.reshape(NHB, 128).T
        miscp[:, NHB + KCT] = float(b2[0])
        qoff = koff = coff = 0
        for s in range(BPC):
            gi = slot_batches[s][c]
            assign[c, s] = gi
            qi, ki = q_idx[gi], k_idx[gi]
            for dc in range(NDC):
                if len(qi):
                    qTp[:, qoff : qoff + len(qi)] = query[
                        gi, qi, dc * 128 : (dc + 1) * 128
                    ].T.astype(ml_dtypes.bfloat16)
                if len(ki):
                    kTp[:, koff : koff + len(ki)] = key[
                        gi, ki, dc * 128 : (dc + 1) * 128
                    ].T.astype(ml_dtypes.bfloat16)
                qoff += QN[s]
                koff += KK[s]
            for kc in range(KC[s]):
                lo, hi = kc * 128, min((kc + 1) * 128, len(ki))
                nrow = max(0, hi - lo)
                if nrow:
                    vap[:nrow, coff * VA : coff * VA + H] = value[gi, ki[lo:hi], :]
                    vap[:nrow, coff * VA + H] = 1.0
                    miscp[:nrow, NHB + coff] = 1.0
                coff += 1
        in_maps.append(
            {
                "qT": qTp,
                "kT": kTp,
                "vaug": vap,
                "w1": W1,
                "zmr": zmr,
                "misc": miscp,
            }
        )
    return in_maps, assign, q_idx, QN, KK


def kernel(query, key, value, q_mask, k_mask, W1, b1, W2, b2):
    in_maps, assign, q_idx, QN, KK = _prep(
        query, key, value, q_mask, k_mask, W1, b1, W2, b2
    )
    nc = _build(QN, KK)
    res = run_bass_kernel_spmd(nc, in_maps, core_ids=list(range(N_CORES)))
    out = np.zeros((B, S1, H), np.float32)
    for c in range(N_CORES):
        yv = res.results[c]["y"]
        for s in range(BPC):
            gi = assign[c, s]
            qi = q_idx[gi]
            if len(qi):
                out[gi, qi, :] = yv[s, : len(qi), :]
    return out


def traced_single_core(query, key, value, q_mask, k_mask, W1, b1, W2, b2, core=0):
    """Run one core's share with NTFF tracing; returns (out, exec_time_ns)."""
    in_maps, assign, q_idx, QN, KK = _prep(
        query, key, value, q_mask, k_mask, W1, b1, W2, b2
    )
    nc = _build(QN, KK)
    tmpdir = os.environ.get("BASS_TRACE_DIR")
    if tmpdir:
        os.makedirs(tmpdir, exist_ok=True)
    res = run_bass_kernel_spmd(
        nc, [in_maps[core]], core_ids=[0], trace=True, tmpdir=tmpdir
    )
    out = np.zeros((B, S1, H), np.float32)
    yv = res.results[0]["y"]
    for s in range(BPC):
        gi = assign[core, s]
        qi = q_idx[gi]
        if len(qi):
            out[gi, qi, :] = yv[s, : len(qi), :]
    return out, res.exec_time_ns

